# revision 24
# baseline (speedup 1.0000x reference)
"""VQ codebook-lookup kernel for TRN2, data-parallel over batch on 8 NeuronCores.

Reference computation (per batch b with class c[b]):
  z = z_e_x[b] viewed as [N=4096, D=256] (D innermost)
  cb = embedding[c[b]*512:(c[b]+1)*512]            # [K=512, D]
  idx[n] = argmin_k ||z[n] - cb[k]||^2 = argmax_k (z[n].cb[k] - ||cb[k]||^2/2)
  out[n] = cb[idx[n]]

Device strategy per core (4 batches):
  - scores S[n,k] via TensorE into PSUM (split-precision passes; see schemes)
  - post="actcp": ACT copies the biased PSUM scores to SBUF (frees the DVE
    from 1x-rate PSUM reads); a DVE tensor_scalar in 2x SBUF mode yields the
    row max; DVE max_index turns the max into the argmax index
  - codewords are fetched by index with a GPSIMD dma_gather straight from a
    bf16 codebook table in DRAM (no one-hot transpose / gather matmuls)
  - input DMAs split across the SP and ACT HWDGE queues (first batch z
    chunked so the first matmuls start early); wrap + out DMAs alternate
    SP/ACT; the last batch emits its wrap/gather/out tail per quarter so
    only ~1/4 of the tail is exposed after the final matmul

Score schemes (argmax must be near-exact: >26 wrong argmaxes fails the
rel-err gate; counts measured on the fixed test inputs, full 32 batches):
  exact3: dots = z1.c1 + z2.c1 + z1.c2, all bf16 (6 matmuls/tile; 4 wrong)
  fp8c:   z1.c1 + z2.c1 bf16 + fp8 DoubleRow z.c2 (12 wrong, rel 0.0132)
  fp8m:   z1.c1 bf16 + fp8 DR z.c2 + fp8 DR z2.(c1hi+c1lo) (13 wrong)
  fp8b:   z1.c1 bf16 + fp8 DR z.c2 + fp8 DR z2.c1 (~19 wrong)
fp8 schemes scale all products by 2^16 so the tiny c2 residuals are
representable in e4m3; argmax is scale-invariant.
bias_dr=1 injects the bias as 4 fp8 terms via one DoubleRow matmul (256 PE
cyc vs 512 for the bf16 hi/lo pair); HW-neutral since PE is not the
bottleneck, so left off.

HW timings (in-NEFF repetition differencing, 8 cores):
  329814 ns  session-start baseline (exact3, everything on SP queue)
  297287 ns  reproduced baseline
  280828 ns  + DMA queue split/chunking + quarter-tail + gpsimd memsets
  258072 ns  + fp8c + ACT score copy (post="actcp")   <- current
  261153 ns  fp8m+bias_dr (PE 150->123us: no gain, DVE-chain bound)
gp_split=1 (row-max tensor_scalar on GPSIMD) fails neuronxcc compile -- the
backend has no gpsimd TensorScalar lowering; do not enable.
CoreSim underestimates HW by ~1.3x: the DVE inter-op pipe-flush (DRAIN ~=
op_dur-266ns, see trainium-docs/engines/02-vector-engine.md) is unmodeled,
making the DVE argmax chain (TS-max + max_index per 128x512 tile) the real
HW bottleneck, not the PE matmuls.
"""

import sys

sys.path.insert(0, "/opt/trn_rl_repo")

import numpy as np

B, D, HH, WW = 32, 256, 64, 64
N = HH * WW            # 4096 positions per batch
K = 512                # codes per class
NUM_CLASSES = 60
NCORES = 8
BPC = B // NCORES      # batches per core
NT = N // 128          # 32 n-tiles per batch

SCHEME = "fp8c"        # "exact3" | "fp8c" | "fp8b" | "fp8m"

_CACHE = {}

# set by test harness to request an NTFF profile
TRACE = False
LAST_EXEC_NS = None


def _build(bpc=BPC, nt=NT, repeat=1, scheme=SCHEME, psum_bufs=4, sco_bufs=5,
           zb_bufs=2, tail="full", mx_bcast="memset", fuse=None,
           tail_eng="act", ms_eng="gp", wrapmode="hw", tailq=1,
           post="actcp", bias_dr=None, gp_split=0, in_sp=0):
    if bias_dr is None:
        bias_dr = BIAS_DR
    # tail: "full" | "nogather" (skip lib load + dma_gather) | "nowrap"
    #       (also skip wrap/replica DMAs) | "nomax" (also skip max_index)
    #       | "nottr" (matmuls + out DMA only)
    # mx_bcast: "stride0" (broadcast AP into max_index) | "memset"
    #       (pre-fill mxt with -inf so in_max is a plain contiguous AP)
    # fuse: how S = psS + bias and its row max are computed
    #   "ttr_psum": one DVE tensor_tensor_reduce reading PSUM (crashes HW)
    #   "ttr_sbuf": ACT copies psS to SBUF, TTR on SBUF operands
    #   "ts_seed":  ACT pre-writes bias into the PSUM bank, matmuls
    #               accumulate on top (BROKEN on HW: matmuls overwrite the
    #               seed where PSUM's has_written bit is clear, and the
    #               ACT write races the first matmul)
    #   "mmbias":   bias accumulated by a ones[2,128] x [bias_hi;bias_lo]
    #               matmul with start=True — race-free, +1024 PE cyc/pair
    if fuse is None:
        fuse = FUSE
    from concourse import bacc, tile, mybir
    import ml_dtypes

    f32 = mybir.dt.float32
    bf16 = mybir.dt.bfloat16
    fp8 = mybir.dt.float8e4
    i16 = mybir.dt.int16
    u16 = mybir.dt.uint16
    Alu = mybir.AluOpType

    from concourse import library_config

    nc = bacc.Bacc("TRN2", target_bir_lowering=False)

    z1_ext = nc.declare_dram_parameter("z1", [bpc, 128, 2, N], bf16, isOutput=False)
    ct1_ext = nc.declare_dram_parameter("ct1", [bpc, 128, 2, K], bf16, isOutput=False)
    if scheme in ("exact3", "fp8c"):
        z2_ext = nc.declare_dram_parameter("z2", [bpc, 128, 2, N], bf16,
                                           isOutput=False)
    if scheme == "exact3":
        ct2_ext = nc.declare_dram_parameter("ct2", [bpc, 128, 2, K], bf16,
                                            isOutput=False)
    if scheme in ("fp8c", "fp8b", "fp8m"):
        z8_ext = nc.declare_dram_parameter("z8", [bpc, 128, 2, N], fp8,
                                           isOutput=False)
        c28_ext = nc.declare_dram_parameter("c28", [bpc, 128, 2, K], fp8,
                                            isOutput=False)
    if scheme in ("fp8b", "fp8m"):
        z28_ext = nc.declare_dram_parameter("z28", [bpc, 128, 2, N], fp8,
                                            isOutput=False)
    if scheme == "fp8b":
        c18_ext = nc.declare_dram_parameter("c18", [bpc, 128, 2, K], fp8,
                                            isOutput=False)
    if scheme == "fp8m":
        c18h_ext = nc.declare_dram_parameter("c18h", [bpc, 128, 2, K], fp8,
                                             isOutput=False)
        c18l_ext = nc.declare_dram_parameter("c18l", [bpc, 128, 2, K], fp8,
                                             isOutput=False)
    if fuse == "mmbias" and bias_dr:
        # bias as 4 fp8 terms laid out [2 part, 2 DR-pair, K]; DoubleRow
        # matmul against a constant 16.0 stationary costs 256 PE cyc
        bias4_ext = nc.declare_dram_parameter("bias4", [bpc, 2, 2, K], fp8,
                                              isOutput=False)
    elif fuse == "mmbias":
        # bias as two bf16 rows (hi, lo) per h-slice, injected via matmul
        bias2_ext = nc.declare_dram_parameter("bias2", [bpc, 2, 2, K], bf16,
                                              isOutput=False)
    else:
        # bias duplicated along axis 2 so both halves of a pair read [128, K]
        bias_ext = nc.declare_dram_parameter("bias", [bpc, 128, 2, K], f32,
                                             isOutput=False)
    # gather table: per-batch class codebook rows, bf16, stays in DRAM
    cbg_ext = nc.declare_dram_parameter("cbg", [bpc, K, D], bf16, isOutput=False)
    out_ext = nc.declare_dram_parameter("out", [bpc, 128, nt, D], bf16,
                                        isOutput=True)
    if tail == "fulldbg":
        dbg_ext = nc.declare_dram_parameter("dbg", [bpc, 128, nt, 8], i16,
                                            isOutput=True)

    if fuse == "mmbias":
        import ml_dtypes as _mld
        if bias_dr:
            ones2_dram = nc.inline_tensor(
                np.full((2, 2, 128), 64.0, dtype=_mld.float8_e4m3),
                name="ones4")
        else:
            ones2_dram = nc.inline_tensor(
                np.ones((2, 128), dtype=_mld.bfloat16), name="ones2")

    teng = None  # resolved inside TileContext: ACT offloads tail DMAs from SP
    if post == "actcp":
        tail_eng = "sp"
    with tile.TileContext(nc) as tc:
        teng = nc.scalar if tail_eng == "act" else nc.sync
        mseng = nc.gpsimd if ms_eng == "gp" else nc.vector
        ieng = nc.sync if in_sp else nc.scalar
        weng2 = nc.scalar
        with (
            tc.tile_pool(name="const", bufs=1) as constp,
            tc.tile_pool(name="zb", bufs=zb_bufs) as zb,
            tc.tile_pool(name="cbp", bufs=2) as cbp,
            tc.tile_pool(name="outp", bufs=2) as outp,
            tc.tile_pool(name="sco", bufs=sco_bufs) as sco,
            tc.tile_pool(name="idxp", bufs=2) as idxp,
            tc.tile_pool(name="psS", bufs=psum_bufs, space="PSUM") as psSp,
        ):
            if fuse == "mmbias":
                if bias_dr:
                    ones2 = constp.tile([2, 2, 128], fp8, tag="ones4")
                else:
                    ones2 = constp.tile([2, 128], bf16, tag="ones2")
                nc.sync.dma_start(ones2[:], ones2_dram[:])
            if post == "actcp":
                strash = constp.tile([128, 2, K], bf16, tag="strash")

            if tail in ("full", "fulldbg", "t4"):
                # Warmup gather: the first dma_gather after the (auto-
                # inserted) mlp ucode library load races its descriptor-gen
                # against library settling on cold start and reads stale
                # indices (measured: corruption only ever in the very first
                # gather of a cold run). Absorb it with a throwaway gather.
                wu_idx = idxp.tile([128, 4], i16, tag="wuidx")
                wu_out = idxp.tile([128, 1, D], bf16, tag="wuout")
                nc.vector.memset(wu_idx[:], 0)
                nc.gpsimd.dma_gather(wu_out[:], cbg_ext[0], wu_idx[:],
                                     num_idxs=64, num_idxs_reg=64,
                                     elem_size=D)

            batches = [bb for _ in range(repeat) for bb in range(bpc)]
            for bi, b in enumerate(batches):
                is_last = bi == len(batches) - 1
                ct1 = cbp.tile([128, 2, K], bf16, tag="ct1")
                # small codebook tensors first: the first matmul needs them
                nc.sync.dma_start(ct1[:], ct1_ext[b])
                if fuse == "mmbias" and bias_dr:
                    bias2t = cbp.tile([2, 2, K], fp8, tag="bias4")
                    nc.sync.dma_start(bias2t[:], bias4_ext[b])
                elif fuse == "mmbias":
                    bias2t = cbp.tile([2, 2, K], bf16, tag="bias2")
                    nc.sync.dma_start(bias2t[:], bias2_ext[b])
                else:
                    bias = cbp.tile([128, 2, K], f32, tag="bias")
                    nc.sync.dma_start(bias[:], bias_ext[b])
                if scheme == "exact3":
                    ct2 = cbp.tile([128, 2, K], bf16, tag="ct2")
                    ieng.dma_start(ct2[:], ct2_ext[b])
                if scheme in ("fp8c", "fp8b", "fp8m"):
                    c28 = cbp.tile([128, 2, K], fp8, tag="c28")
                    nc.sync.dma_start(c28[:], c28_ext[b])
                if scheme == "fp8b":
                    c18 = cbp.tile([128, 2, K], fp8, tag="c18")
                    nc.sync.dma_start(c18[:], c18_ext[b])
                if scheme == "fp8m":
                    c18h = cbp.tile([128, 2, K], fp8, tag="c18h")
                    nc.sync.dma_start(c18h[:], c18h_ext[b])
                    c18l = cbp.tile([128, 2, K], fp8, tag="c18l")
                    nc.sync.dma_start(c18l[:], c18l_ext[b])
                z1 = zb.tile([128, 2, N], bf16, tag="z1")
                if b == 0:
                    # chunked: first matmuls can start before the full load
                    nc.sync.dma_start(z1[:, :, 0:1024], z1_ext[b][:, :, 0:1024])
                    nc.sync.dma_start(z1[:, :, 1024:N], z1_ext[b][:, :, 1024:N])
                else:
                    nc.sync.dma_start(z1[:], z1_ext[b])
                if scheme in ("exact3", "fp8c"):
                    z2 = zb.tile([128, 2, N], bf16, tag="z2")
                    if b == 0:
                        ieng.dma_start(z2[:, :, 0:1024],
                                       z2_ext[b][:, :, 0:1024])
                        ieng.dma_start(z2[:, :, 1024:N],
                                       z2_ext[b][:, :, 1024:N])
                    else:
                        ieng.dma_start(z2[:], z2_ext[b])
                if scheme in ("fp8c", "fp8b", "fp8m"):
                    z8 = zb.tile([128, 2, N], fp8, tag="z8")
                    ieng.dma_start(z8[:], z8_ext[b])
                if scheme in ("fp8b", "fp8m"):
                    z28 = zb.tile([128, 2, N], fp8, tag="z28")
                    nc.sync.dma_start(z28[:], z28_ext[b])

                idx8t = idxp.tile([128, nt, 8], u16, tag="idx8")
                idxw = idxp.tile([128, nt, 8], i16, tag="idxw")
                if tail in ("full", "fulldbg", "nogather", "t3", "t4"):
                    # partitions 32:128 are never wrap-written (HW reads
                    # 16:32, sim 0:16) — zero the whole tile first so the
                    # gather's full-span idxs view is initialized
                    mseng.memset(idxw[:], 0)
                out_sb = outp.tile([128, nt, D], bf16, tag="out")

                # two n-tiles per iteration (psS spans 2 PSUM banks)
                for p in range(nt // 2):
                    psS = psSp.tile([128, 2, K], f32, tag="psS")
                    seeded = fuse == "ts_seed" and tail != "nottr"
                    if seeded:
                        nc.scalar.copy(psS[:], bias[:])
                    for h in range(2):
                        n0 = (2 * p + h) * 128
                        if fuse == "mmbias" and bias_dr:
                            nc.tensor.matmul(
                                psS[:, h, :], ones2[:], bias2t[:],
                                start=True, stop=False,
                                perf_mode=mybir.MatmulPerfMode.DoubleRow)
                        elif fuse == "mmbias":
                            nc.tensor.matmul(psS[:, h, :], ones2[:],
                                             bias2t[:, h, :], start=True,
                                             stop=False)
                        if scheme == "exact3":
                            mms = [(z1, ct1, 0), (z1, ct1, 1), (z2, ct1, 0),
                                   (z2, ct1, 1), (z1, ct2, 0), (z1, ct2, 1)]
                        elif scheme == "fp8c":
                            mms = [(z1, ct1, 0), (z1, ct1, 1), (z2, ct1, 0),
                                   (z2, ct1, 1)]
                        else:
                            mms = [(z1, ct1, 0), (z1, ct1, 1)]
                        ndr = {"fp8c": 1, "fp8b": 2, "fp8m": 3,
                               "exact3": 0}[scheme]
                        ntot = len(mms) + ndr
                        for i, (za, ca, cd) in enumerate(mms):
                            st = (i == 0 and not seeded
                                  and fuse != "mmbias")
                            nc.tensor.matmul(psS[:, h, :], za[:, cd, n0:n0 + 128],
                                             ca[:, cd, :], start=st,
                                             stop=(i == ntot - 1),
                                             skip_group_check=seeded)
                        if scheme in ("fp8c", "fp8b", "fp8m"):
                            nc.tensor.matmul(
                                psS[:, h, :], z8[:, :, n0:n0 + 128], c28[:],
                                start=False, stop=(len(mms) + 1 == ntot),
                                perf_mode=mybir.MatmulPerfMode.DoubleRow,
                                skip_group_check=seeded)
                        if scheme == "fp8b":
                            nc.tensor.matmul(
                                psS[:, h, :], z28[:, :, n0:n0 + 128], c18[:],
                                start=False, stop=True,
                                perf_mode=mybir.MatmulPerfMode.DoubleRow,
                                skip_group_check=seeded)
                        if scheme == "fp8m":
                            nc.tensor.matmul(
                                psS[:, h, :], z28[:, :, n0:n0 + 128], c18h[:],
                                start=False, stop=False,
                                perf_mode=mybir.MatmulPerfMode.DoubleRow,
                                skip_group_check=seeded)
                            nc.tensor.matmul(
                                psS[:, h, :], z28[:, :, n0:n0 + 128], c18l[:],
                                start=False, stop=True,
                                perf_mode=mybir.MatmulPerfMode.DoubleRow,
                                skip_group_check=seeded)

                    # S = dots + bias (SBUF copy) and row-max, in one DVE op;
                    # then the argmax index of each row via max_index
                    if tail == "t0":
                        continue
                    if tail == "nottr":
                        nc.scalar.copy(out_sb[:, 2 * p:2 * p + 2, :],
                                       psS[:, :, 0:D])
                        continue
                    S_sb = sco.tile([128, 2, K], f32, tag="S")
                    mxt = sco.tile([128, 2, 8], f32, tag="mx")
                    post_done = False
                    if mx_bcast == "memset":
                        mseng.memset(mxt[:], -3.0e38)
                    if fuse in ("ts_seed", "mmbias") and post == "actcp":
                        # ACT moves biased scores to SBUF per h (copy h0
                        # overlaps the h1 matmuls); DVE max runs in 2x mode
                        # on SBUF operands (PSUM reads are 1x).  out goes to
                        # a bf16 trash tile (not in-place) to avoid
                        # same-address read/write pipe hazards.  DVE order
                        # TS0,MI0,TS1,MI1 starts each index scan as soon as
                        # its max lands.
                        for h in range(2):
                            nc.scalar.copy(S_sb[:, h, :], psS[:, h, :])
                            meng = nc.gpsimd if (gp_split and h == 0) \
                                else nc.vector
                            meng.tensor_scalar(
                                out=strash[:, h, :], in0=S_sb[:, h, :],
                                scalar1=0.0, scalar2=None, op0=Alu.add,
                                op1=Alu.max, accum_out=mxt[:, h, 0:1])
                            in_max = (mxt[:, h, 0:1].broadcast_to([128, 8])
                                      if mx_bcast == "stride0"
                                      else mxt[:, h, :])
                            nc.vector.max_index(out=idx8t[:, 2 * p + h, :],
                                                in_max=in_max,
                                                in_values=S_sb[:, h, :])
                        post_done = True
                    elif fuse in ("ts_seed", "mmbias"):
                        # psS already biased; copy to SBUF + row max in one op
                        for h in range(2):
                            nc.vector.tensor_scalar(
                                out=S_sb[:, h, :], in0=psS[:, h, :],
                                scalar1=0.0, scalar2=None, op0=Alu.add,
                                op1=Alu.max, accum_out=mxt[:, h, 0:1])
                    elif fuse == "ttr_sbuf":
                        Snb = sco.tile([128, 2, K], f32, tag="Snb")
                        nc.scalar.copy(Snb[:], psS[:])
                        for h in range(2):
                            nc.vector.tensor_tensor_reduce(
                                out=S_sb[:, h, :], in0=Snb[:, h, :],
                                in1=bias[:, h, :], scale=1.0, scalar=-3.0e38,
                                op0=Alu.add, op1=Alu.max,
                                accum_out=mxt[:, h, 0:1])
                    else:
                        for h in range(2):
                            nc.vector.tensor_tensor_reduce(
                                out=S_sb[:, h, :], in0=psS[:, h, :],
                                in1=bias[:, h, :], scale=1.0, scalar=-3.0e38,
                                op0=Alu.add, op1=Alu.max,
                                accum_out=mxt[:, h, 0:1])
                    if tail == "t1":
                        continue
                    if tail == "nomax":
                        nc.scalar.copy(out_sb[:, 2 * p:2 * p + 2, :],
                                       S_sb[:, :, 0:D])
                        continue
                    if not post_done:
                        for h in range(2):
                            in_max = (mxt[:, h, 0:1].broadcast_to([128, 8])
                                      if mx_bcast == "stride0"
                                      else mxt[:, h, :])
                            nc.vector.max_index(out=idx8t[:, 2 * p + h, :],
                                                in_max=in_max,
                                                in_values=S_sb[:, h, :])
                    if tail == "t2":
                        continue

                    if (tailq and is_last and tail in ("full", "t4")
                            and p % 4 == 3):
                        qq = p // 4
                        ts = slice(8 * qq, 8 * (qq + 1))
                        for pp in range(8):
                            s8 = idx8t[16 * pp:16 * (pp + 1), ts, 0:1] \
                                .bitcast(i16)
                            weng = weng2 if pp % 2 else nc.sync
                            if wrapmode in ("both", "hw"):
                                weng.dma_start(idxw[16:32, ts, pp:pp + 1], s8)
                            if wrapmode in ("both", "sim"):
                                weng.dma_start(idxw[0:16, ts, pp:pp + 1], s8)
                        nc.gpsimd.dma_gather(
                            out_sb[:, ts, :], cbg_ext[b], idxw[:, ts, :],
                            num_idxs=N // 4, num_idxs_reg=N // 4, elem_size=D)
                        oeng = nc.sync if qq % 2 else weng2
                        oeng.dma_start(out_ext[b][:, ts], out_sb[:, ts])

                if (not (tailq and is_last)
                        and tail in ("full", "fulldbg", "nogather", "t3",
                                     "t4")):
                    # wrap: idxs[i%16, i//16] = idx of position i
                    # (i = t*128 + pk  ->  [pk%16, t*8 + pk//16]).
                    # The SWDGE queue-0 descgen core reads partitions 16:32
                    # (measured); CoreSim reads 0:16 — write both copies
                    # directly from idx8t.
                    for pp in range(8):
                        s8 = idx8t[16 * pp:16 * (pp + 1), :, 0:1].bitcast(i16)
                        weng = weng2 if pp % 2 else nc.sync
                        if wrapmode in ("both", "hw"):
                            weng.dma_start(idxw[16:32, :, pp:pp + 1], s8)
                        if wrapmode in ("both", "sim"):
                            weng.dma_start(idxw[0:16, :, pp:pp + 1], s8)
                    if tail in ("full", "fulldbg", "t4"):
                        # dma_gather caps out between 1024 and 2048 idxs
                        # per instruction (measured) — split in 4
                        for qq in range(4):
                            ts = slice(8 * qq, 8 * (qq + 1))
                            nc.gpsimd.dma_gather(
                                out_sb[:, ts, :], cbg_ext[b],
                                idxw[:, ts, :], num_idxs=N // 4,
                                num_idxs_reg=N // 4, elem_size=D)

                if tail in ("t0", "t1", "t2", "t3"):
                    # same-size dump of z1 (out_sb is never written here)
                    teng.dma_start(out_ext[b], z1[:])
                elif tailq and is_last and tail in ("full", "t4"):
                    pass  # emitted per quarter inside the p loop
                elif tail in ("full", "fulldbg", "t4"):
                    hn = nt // 2
                    nc.sync.dma_start(out_ext[b][:, 0:hn], out_sb[:, 0:hn])
                    weng2.dma_start(out_ext[b][:, hn:nt], out_sb[:, hn:nt])
                    if tail == "fulldbg":
                        teng.dma_start(dbg_ext[b], idxw[:])
                elif tail in ("nottr", "nomax"):
                    teng.dma_start(out_ext[b], out_sb[:])
                elif tail == "nowrap":
                    teng.dma_start(out_ext[b][:, :, 0:8].bitcast(u16),
                                   idx8t[:, :, :])
                else:  # nogather
                    teng.dma_start(out_ext[b][:, :, 0:8].bitcast(i16),
                                   idxw[:, :, :])

    nc.compile()
    return nc


def _get_nc():
    if "nc" not in _CACHE:
        _CACHE["nc"] = _build(fuse=FUSE)
    return _CACHE["nc"]


def _prep_in_maps(z_e_x, c, embedding, scheme=None, bpc=BPC,
                  fuse="mmbias", bias_dr=None):
    if scheme is None:
        scheme = SCHEME
    if bias_dr is None:
        bias_dr = BIAS_DR
    import ml_dtypes

    bf = ml_dtypes.bfloat16
    f8 = ml_dtypes.float8_e4m3

    z = np.ascontiguousarray(np.asarray(z_e_x), dtype=np.float32)      # [B, D, H, W]
    cls = np.asarray(c).astype(np.int64)                               # [B]
    emb = np.ascontiguousarray(np.asarray(embedding), dtype=np.float32)

    def dchunk(a):  # [B, 256, X] -> [B, 128, 2, X] with d = cd*128 + p
        return np.ascontiguousarray(
            a.reshape(B, 2, 128, a.shape[-1]).transpose(0, 2, 1, 3))

    zf = z.reshape(B, D, N)                                            # [B, 256, 4096]
    z1 = zf.astype(bf)
    z2f = zf - z1.astype(np.float32)

    cb = emb.reshape(NUM_CLASSES, K, D)[cls]                           # [B, 512, 256]
    cbT = np.ascontiguousarray(cb.transpose(0, 2, 1))                  # [B, 256, 512]
    c1 = cbT.astype(bf)
    c2f = cbT - c1.astype(np.float32)

    scale = (np.float32(65536.0) if scheme in ("fp8c", "fp8b", "fp8m")
             else np.float32(1.0))

    bias = -0.5 * np.sum(cb.astype(np.float64) ** 2, axis=2)           # [B, 512]
    bias_s = (bias * np.float64(scale)).astype(np.float32)
    if fuse == "mmbias" and bias_dr:
        assert scale > 1.0, "bias_dr needs the fp8 2^16 scaling"
        terms = []
        r = bias_s.astype(np.float32).copy()
        for _ in range(4):
            t = (r / np.float32(64.0)).astype(f8)
            terms.append(t)
            r = r - np.float32(64.0) * t.astype(np.float32)
        # [B, 2(part), 2(pair), K]
        bias4 = np.ascontiguousarray(
            np.stack(terms, axis=1).reshape(B, 2, 2, K))
    elif fuse == "mmbias":
        bhi = bias_s.astype(bf)
        blo = (bias_s - bhi.astype(np.float32)).astype(bf)
        # [B, 2(hi/lo), 2(h), K]
        bias2 = np.ascontiguousarray(
            np.stack([bhi, blo], axis=1)[:, :, None, :].repeat(2, axis=2))
    else:
        bias_bc = np.ascontiguousarray(np.broadcast_to(
            bias_s[:, None, None, :], (B, 128, 2, K)))

    cbg = cb.astype(bf)                                                # [B, 512, 256]

    per = {
        "z1": dchunk(z1),
        "ct1": dchunk((c1.astype(np.float32) * scale).astype(bf)),
        "cbg": cbg,
    }
    if fuse == "mmbias" and bias_dr:
        per["bias4"] = bias4
    elif fuse == "mmbias":
        per["bias2"] = bias2
    else:
        per["bias"] = bias_bc
    if scheme in ("exact3", "fp8c"):
        per["z2"] = dchunk(z2f.astype(bf))
    if scheme == "exact3":
        per["ct2"] = dchunk(c2f.astype(bf))
    if scheme in ("fp8c", "fp8b", "fp8m"):
        per["z8"] = dchunk((zf * np.float32(16.0)).astype(f8))
        per["c28"] = dchunk((c2f * np.float32(4096.0)).astype(f8))
    if scheme in ("fp8b", "fp8m"):
        per["z28"] = dchunk((z2f * np.float32(256.0)).astype(f8))
    if scheme == "fp8b":
        per["c18"] = dchunk((c1.astype(np.float32) * np.float32(256.0)).astype(f8))
    if scheme == "fp8m":
        c1s = c1.astype(np.float32) * np.float32(256.0)
        c18h = c1s.astype(f8)
        c18l = (c1s - c18h.astype(np.float32)).astype(f8)
        per["c18h"] = dchunk(c18h)
        per["c18l"] = dchunk(c18l)

    in_maps = []
    for i in range(NCORES):
        s = slice(i * bpc, (i + 1) * bpc)
        in_maps.append({k: v[s] for k, v in per.items()})
    return in_maps


FUSE = "mmbias"
BIAS_DR = 0


def kernel(z_e_x, c, embedding):
    from concourse.bass_utils import run_bass_kernel_spmd

    global LAST_EXEC_NS

    in_maps = _prep_in_maps(z_e_x, c, embedding, fuse=FUSE)
    nc = _get_nc()
    res = run_bass_kernel_spmd(nc, in_maps, core_ids=list(range(NCORES)),
                               trace=TRACE)
    LAST_EXEC_NS = res.exec_time_ns

    outs = np.concatenate([res.results[i]["out"].astype(np.float32)
                           for i in range(NCORES)], axis=0)
    # [B, 128, NT, D] -> [B, N, D] with n = t*128 + p
    out = outs.transpose(0, 2, 1, 3).reshape(B, N, D)
    return np.ascontiguousarray(out.reshape(B, HH, WW, D))



# revision 30
# speedup vs baseline: 1.1563x; 1.1563x over previous
"""VQ codebook-lookup kernel for TRN2, data-parallel over batch on 8 NeuronCores.

Reference computation (per batch b with class c[b]):
  z = z_e_x[b] viewed as [N=4096, D=256] (D innermost)
  cb = embedding[c[b]*512:(c[b]+1)*512]            # [K=512, D]
  idx[n] = argmin_k ||z[n] - cb[k]||^2 = argmax_k (z[n].cb[k] - ||cb[k]||^2/2)
  out[n] = cb[idx[n]]

Device strategy per core (4 batches):
  - scores S[n,k] via TensorE into PSUM (split-precision passes; see schemes)
  - post="actcp": ACT copies the biased PSUM scores to SBUF (frees the DVE
    from 1x-rate PSUM reads); a DVE tensor_scalar in 2x SBUF mode yields the
    row max; DVE max_index turns the max into the argmax index
  - codewords are fetched by index with a GPSIMD dma_gather straight from a
    bf16 codebook table in DRAM (no one-hot transpose / gather matmuls)
  - input DMAs split across the SP and ACT HWDGE queues (first batch z
    chunked so the first matmuls start early); wrap + out DMAs alternate
    SP/ACT; the last batch emits its wrap/gather/out tail per quarter so
    only ~1/4 of the tail is exposed after the final matmul

Score schemes (argmax must be near-exact: >26 wrong argmaxes fails the
rel-err gate; counts measured on the fixed test inputs, full 32 batches):
  exact3: dots = z1.c1 + z2.c1 + z1.c2, all bf16 (6 matmuls/tile; 4 wrong)
  fp8c:   z1.c1 + z2.c1 bf16 + fp8 DoubleRow z.c2 (12 wrong, rel 0.0132)
  fp8m:   z1.c1 bf16 + fp8 DR z.c2 + fp8 DR z2.(c1hi+c1lo) (13 wrong)
  fp8b:   z1.c1 bf16 + fp8 DR z.c2 + fp8 DR z2.c1 (~19 wrong)
fp8 schemes scale all products by 2^16 so the tiny c2 residuals are
representable in e4m3; argmax is scale-invariant.
bias_dr=1 injects the bias as 4 fp8 terms via one DoubleRow matmul (256 PE
cyc vs 512 for the bf16 hi/lo pair); HW-neutral since PE is not the
bottleneck, so left off.

HW timings (in-NEFF repetition differencing, 8 cores):
  329814 ns  session-start baseline (exact3, everything on SP queue)
  297287 ns  reproduced baseline
  280828 ns  + DMA queue split/chunking + quarter-tail + gpsimd memsets
  258072 ns  + fp8c + ACT score copy (post="actcp")   <- current
  261153 ns  fp8m+bias_dr (PE 150->123us: no gain, DVE-chain bound)
gp_split=1 (row-max tensor_scalar on GPSIMD) fails neuronxcc compile -- the
backend has no gpsimd TensorScalar lowering; do not enable.
CoreSim underestimates HW by ~1.3x: the DVE inter-op pipe-flush (DRAIN ~=
op_dur-266ns, see trainium-docs/engines/02-vector-engine.md) is unmodeled,
making the DVE argmax chain (TS-max + max_index per 128x512 tile) the real
HW bottleneck, not the PE matmuls.

Failed experiments (do not retry):
  337375 ns  per-h ACT copies + TS,MI,TS,MI order + sco_bufs=5: ScalarE
             pays a large per-op bubble, 128 small copies >> 64 big ones
  sim-worse  prefetch emission of next batch's inputs (all queues tried):
             SP issue bandwidth is the binding constraint; the validated
             SP/ACT split is locally optimal in CoreSim and CoreSim's
             config ordering has matched HW every time it was tested
  sim-worse  z loads via gpsimd SWDGE dma_start (Pool DMA issue is priced
             ~5us/2MB, collides with gathers)
  no 2x mode InstMaxIndex costs (58+FD) cyc regardless of dtype (no
             2x_1p uop), so the argmax floor is 2 DVE passes per tile
  no max-acc ACT activation accum_out is sum-only; tensor_scalar accum_out
             must be [128,1] (no segmented accum); fold-DMA APs (partition
             split src) fail the interp's conflict checker
Flags prefetch=1 / zdma=1 / in_sp=1 keep those experiments reproducible.
"""

import sys

sys.path.insert(0, "/opt/trn_rl_repo")

import numpy as np

B, D, HH, WW = 32, 256, 64, 64
N = HH * WW            # 4096 positions per batch
K = 512                # codes per class
NUM_CLASSES = 60
NCORES = 8
BPC = B // NCORES      # batches per core
NT = N // 128          # 32 n-tiles per batch

SCHEME = "fp8c"        # "exact3" | "fp8c" | "fp8b" | "fp8m"

_CACHE = {}

# set by test harness to request an NTFF profile
TRACE = False
LAST_EXEC_NS = None


def _build(bpc=BPC, nt=NT, repeat=1, scheme=SCHEME, psum_bufs=4, sco_bufs=3,
           zb_bufs=2, tail="full", mx_bcast="memset", fuse=None,
           tail_eng="act", ms_eng="gp", wrapmode="hw", tailq=1,
           post="actcp", bias_dr=None, gp_split=0, in_sp=0, prefetch=0,
           zdma=0):
    if bias_dr is None:
        bias_dr = BIAS_DR
    # tail: "full" | "nogather" (skip lib load + dma_gather) | "nowrap"
    #       (also skip wrap/replica DMAs) | "nomax" (also skip max_index)
    #       | "nottr" (matmuls + out DMA only)
    # mx_bcast: "stride0" (broadcast AP into max_index) | "memset"
    #       (pre-fill mxt with -inf so in_max is a plain contiguous AP)
    # fuse: how S = psS + bias and its row max are computed
    #   "ttr_psum": one DVE tensor_tensor_reduce reading PSUM (crashes HW)
    #   "ttr_sbuf": ACT copies psS to SBUF, TTR on SBUF operands
    #   "ts_seed":  ACT pre-writes bias into the PSUM bank, matmuls
    #               accumulate on top (BROKEN on HW: matmuls overwrite the
    #               seed where PSUM's has_written bit is clear, and the
    #               ACT write races the first matmul)
    #   "mmbias":   bias accumulated by a ones[2,128] x [bias_hi;bias_lo]
    #               matmul with start=True — race-free, +1024 PE cyc/pair
    if fuse is None:
        fuse = FUSE
    from concourse import bacc, tile, mybir
    import ml_dtypes

    f32 = mybir.dt.float32
    bf16 = mybir.dt.bfloat16
    fp8 = mybir.dt.float8e4
    i16 = mybir.dt.int16
    u16 = mybir.dt.uint16
    Alu = mybir.AluOpType

    from concourse import library_config

    nc = bacc.Bacc("TRN2", target_bir_lowering=False)

    z1_ext = nc.declare_dram_parameter("z1", [bpc, 128, 2, N], bf16, isOutput=False)
    ct1_ext = nc.declare_dram_parameter("ct1", [bpc, 128, 2, K], bf16, isOutput=False)
    if scheme in ("exact3", "fp8c"):
        z2_ext = nc.declare_dram_parameter("z2", [bpc, 128, 2, N], bf16,
                                           isOutput=False)
    if scheme == "exact3":
        ct2_ext = nc.declare_dram_parameter("ct2", [bpc, 128, 2, K], bf16,
                                            isOutput=False)
    if scheme in ("fp8c", "fp8b", "fp8m"):
        z8_ext = nc.declare_dram_parameter("z8", [bpc, 128, 2, N], fp8,
                                           isOutput=False)
        c28_ext = nc.declare_dram_parameter("c28", [bpc, 128, 2, K], fp8,
                                            isOutput=False)
    if scheme in ("fp8b", "fp8m"):
        z28_ext = nc.declare_dram_parameter("z28", [bpc, 128, 2, N], fp8,
                                            isOutput=False)
    if scheme == "fp8b":
        c18_ext = nc.declare_dram_parameter("c18", [bpc, 128, 2, K], fp8,
                                            isOutput=False)
    if scheme == "fp8m":
        c18h_ext = nc.declare_dram_parameter("c18h", [bpc, 128, 2, K], fp8,
                                             isOutput=False)
        c18l_ext = nc.declare_dram_parameter("c18l", [bpc, 128, 2, K], fp8,
                                             isOutput=False)
    if fuse == "mmbias" and bias_dr:
        # bias as 4 fp8 terms laid out [2 part, 2 DR-pair, K]; DoubleRow
        # matmul against a constant 16.0 stationary costs 256 PE cyc
        bias4_ext = nc.declare_dram_parameter("bias4", [bpc, 2, 2, K], fp8,
                                              isOutput=False)
    elif fuse == "mmbias":
        # bias as two bf16 rows (hi, lo) per h-slice, injected via matmul
        bias2_ext = nc.declare_dram_parameter("bias2", [bpc, 2, 2, K], bf16,
                                              isOutput=False)
    else:
        # bias duplicated along axis 2 so both halves of a pair read [128, K]
        bias_ext = nc.declare_dram_parameter("bias", [bpc, 128, 2, K], f32,
                                             isOutput=False)
    # gather table: per-batch class codebook rows, bf16, stays in DRAM
    cbg_ext = nc.declare_dram_parameter("cbg", [bpc, K, D], bf16, isOutput=False)
    out_ext = nc.declare_dram_parameter("out", [bpc, 128, nt, D], bf16,
                                        isOutput=True)
    if tail == "fulldbg":
        dbg_ext = nc.declare_dram_parameter("dbg", [bpc, 128, nt, 8], i16,
                                            isOutput=True)

    if fuse == "mmbias":
        import ml_dtypes as _mld
        if bias_dr:
            ones2_dram = nc.inline_tensor(
                np.full((2, 2, 128), 64.0, dtype=_mld.float8_e4m3),
                name="ones4")
        else:
            ones2_dram = nc.inline_tensor(
                np.ones((2, 128), dtype=_mld.bfloat16), name="ones2")

    teng = None  # resolved inside TileContext: ACT offloads tail DMAs from SP
    if post == "actcp":
        tail_eng = "sp"
    with tile.TileContext(nc) as tc:
        teng = nc.scalar if tail_eng == "act" else nc.sync
        mseng = nc.gpsimd if ms_eng == "gp" else nc.vector
        ieng = nc.sync if (in_sp or prefetch) else nc.scalar
        weng2 = nc.sync if prefetch else nc.scalar
        zeng = nc.gpsimd if zdma else ieng
        z1eng = nc.gpsimd if zdma else nc.sync
        with (
            tc.tile_pool(name="const", bufs=1) as constp,
            tc.tile_pool(name="zb", bufs=zb_bufs) as zb,
            tc.tile_pool(name="cbp", bufs=2) as cbp,
            tc.tile_pool(name="outp", bufs=2) as outp,
            tc.tile_pool(name="sco", bufs=sco_bufs) as sco,
            tc.tile_pool(name="idxp", bufs=2) as idxp,
            tc.tile_pool(name="psS", bufs=psum_bufs, space="PSUM") as psSp,
        ):
            if fuse == "mmbias":
                if bias_dr:
                    ones2 = constp.tile([2, 2, 128], fp8, tag="ones4")
                else:
                    ones2 = constp.tile([2, 128], bf16, tag="ones2")
                nc.sync.dma_start(ones2[:], ones2_dram[:])
            if post == "actcp":
                strash = constp.tile([128, 2, K], bf16, tag="strash")

            if tail in ("full", "fulldbg", "t4"):
                # Warmup gather: the first dma_gather after the (auto-
                # inserted) mlp ucode library load races its descriptor-gen
                # against library settling on cold start and reads stale
                # indices (measured: corruption only ever in the very first
                # gather of a cold run). Absorb it with a throwaway gather.
                wu_idx = idxp.tile([128, 4], i16, tag="wuidx")
                wu_out = idxp.tile([128, 1, D], bf16, tag="wuout")
                nc.vector.memset(wu_idx[:], 0)
                nc.gpsimd.dma_gather(wu_out[:], cbg_ext[0], wu_idx[:],
                                     num_idxs=64, num_idxs_reg=64,
                                     elem_size=D)

            def emit_inputs(b, first):
                # all input issue on SP; with prefetch the next batch's
                # inputs are emitted BEFORE this batch's tail DMAs so the
                # ACT queue carries only score copies (no head-of-line
                # blocking of the copy stream at batch boundaries)
                t = {}
                ct1 = cbp.tile([128, 2, K], bf16, tag="ct1")
                # small codebook tensors first: the first matmul needs them
                nc.sync.dma_start(ct1[:], ct1_ext[b])
                t["ct1"] = ct1
                if fuse == "mmbias" and bias_dr:
                    bias2t = cbp.tile([2, 2, K], fp8, tag="bias4")
                    nc.sync.dma_start(bias2t[:], bias4_ext[b])
                    t["bias2t"] = bias2t
                elif fuse == "mmbias":
                    bias2t = cbp.tile([2, 2, K], bf16, tag="bias2")
                    nc.sync.dma_start(bias2t[:], bias2_ext[b])
                    t["bias2t"] = bias2t
                else:
                    bias = cbp.tile([128, 2, K], f32, tag="bias")
                    nc.sync.dma_start(bias[:], bias_ext[b])
                    t["bias"] = bias
                if scheme == "exact3":
                    ct2 = cbp.tile([128, 2, K], bf16, tag="ct2")
                    ieng.dma_start(ct2[:], ct2_ext[b])
                    t["ct2"] = ct2
                if scheme in ("fp8c", "fp8b", "fp8m"):
                    c28 = cbp.tile([128, 2, K], fp8, tag="c28")
                    nc.sync.dma_start(c28[:], c28_ext[b])
                    t["c28"] = c28
                if scheme == "fp8b":
                    c18 = cbp.tile([128, 2, K], fp8, tag="c18")
                    nc.sync.dma_start(c18[:], c18_ext[b])
                    t["c18"] = c18
                if scheme == "fp8m":
                    c18h = cbp.tile([128, 2, K], fp8, tag="c18h")
                    nc.sync.dma_start(c18h[:], c18h_ext[b])
                    t["c18h"] = c18h
                    c18l = cbp.tile([128, 2, K], fp8, tag="c18l")
                    nc.sync.dma_start(c18l[:], c18l_ext[b])
                    t["c18l"] = c18l
                z1 = zb.tile([128, 2, N], bf16, tag="z1")
                if first:
                    # chunked: first matmuls can start before the full load
                    z1eng.dma_start(z1[:, :, 0:1024], z1_ext[b][:, :, 0:1024])
                    z1eng.dma_start(z1[:, :, 1024:N], z1_ext[b][:, :, 1024:N])
                else:
                    z1eng.dma_start(z1[:], z1_ext[b])
                t["z1"] = z1
                if scheme in ("exact3", "fp8c"):
                    z2 = zb.tile([128, 2, N], bf16, tag="z2")
                    if first:
                        zeng.dma_start(z2[:, :, 0:1024],
                                       z2_ext[b][:, :, 0:1024])
                        zeng.dma_start(z2[:, :, 1024:N],
                                       z2_ext[b][:, :, 1024:N])
                    else:
                        zeng.dma_start(z2[:], z2_ext[b])
                    t["z2"] = z2
                if scheme in ("fp8c", "fp8b", "fp8m"):
                    z8 = zb.tile([128, 2, N], fp8, tag="z8")
                    zeng.dma_start(z8[:], z8_ext[b])
                    t["z8"] = z8
                if scheme in ("fp8b", "fp8m"):
                    z28 = zb.tile([128, 2, N], fp8, tag="z28")
                    zeng.dma_start(z28[:], z28_ext[b])
                    t["z28"] = z28
                return t

            batches = [bb for _ in range(repeat) for bb in range(bpc)]
            nxt = emit_inputs(batches[0], True)
            for bi, b in enumerate(batches):
                is_last = bi == len(batches) - 1
                if not prefetch and bi > 0:
                    nxt = emit_inputs(b, False)
                cur = nxt
                ct1 = cur["ct1"]
                bias2t = cur.get("bias2t")
                bias = cur.get("bias")
                ct2 = cur.get("ct2")
                c28 = cur.get("c28")
                c18 = cur.get("c18")
                c18h = cur.get("c18h")
                c18l = cur.get("c18l")
                z1 = cur["z1"]
                z2 = cur.get("z2")
                z8 = cur.get("z8")
                z28 = cur.get("z28")
                if not prefetch and not is_last:
                    nxt = None

                idx8t = idxp.tile([128, nt, 8], u16, tag="idx8")
                idxw = idxp.tile([128, nt, 8], i16, tag="idxw")
                if tail in ("full", "fulldbg", "nogather", "t3", "t4"):
                    # partitions 32:128 are never wrap-written (HW reads
                    # 16:32, sim 0:16) — zero the whole tile first so the
                    # gather's full-span idxs view is initialized
                    mseng.memset(idxw[:], 0)
                out_sb = outp.tile([128, nt, D], bf16, tag="out")

                # two n-tiles per iteration (psS spans 2 PSUM banks)
                for p in range(nt // 2):
                    if prefetch and not is_last and p == 8:
                        # prefetch the next batch's inputs mid-batch: early
                        # enough to land before they're needed, late enough
                        # not to stall this batch's own chunked loads
                        nxt = emit_inputs(batches[bi + 1], False)
                    psS = psSp.tile([128, 2, K], f32, tag="psS")
                    seeded = fuse == "ts_seed" and tail != "nottr"
                    if seeded:
                        nc.scalar.copy(psS[:], bias[:])
                    for h in range(2):
                        n0 = (2 * p + h) * 128
                        if fuse == "mmbias" and bias_dr:
                            nc.tensor.matmul(
                                psS[:, h, :], ones2[:], bias2t[:],
                                start=True, stop=False,
                                perf_mode=mybir.MatmulPerfMode.DoubleRow)
                        elif fuse == "mmbias":
                            nc.tensor.matmul(psS[:, h, :], ones2[:],
                                             bias2t[:, h, :], start=True,
                                             stop=False)
                        if scheme == "exact3":
                            mms = [(z1, ct1, 0), (z1, ct1, 1), (z2, ct1, 0),
                                   (z2, ct1, 1), (z1, ct2, 0), (z1, ct2, 1)]
                        elif scheme == "fp8c":
                            mms = [(z1, ct1, 0), (z1, ct1, 1), (z2, ct1, 0),
                                   (z2, ct1, 1)]
                        else:
                            mms = [(z1, ct1, 0), (z1, ct1, 1)]
                        ndr = {"fp8c": 1, "fp8b": 2, "fp8m": 3,
                               "exact3": 0}[scheme]
                        ntot = len(mms) + ndr
                        for i, (za, ca, cd) in enumerate(mms):
                            st = (i == 0 and not seeded
                                  and fuse != "mmbias")
                            nc.tensor.matmul(psS[:, h, :], za[:, cd, n0:n0 + 128],
                                             ca[:, cd, :], start=st,
                                             stop=(i == ntot - 1),
                                             skip_group_check=seeded)
                        if scheme in ("fp8c", "fp8b", "fp8m"):
                            nc.tensor.matmul(
                                psS[:, h, :], z8[:, :, n0:n0 + 128], c28[:],
                                start=False, stop=(len(mms) + 1 == ntot),
                                perf_mode=mybir.MatmulPerfMode.DoubleRow,
                                skip_group_check=seeded)
                        if scheme == "fp8b":
                            nc.tensor.matmul(
                                psS[:, h, :], z28[:, :, n0:n0 + 128], c18[:],
                                start=False, stop=True,
                                perf_mode=mybir.MatmulPerfMode.DoubleRow,
                                skip_group_check=seeded)
                        if scheme == "fp8m":
                            nc.tensor.matmul(
                                psS[:, h, :], z28[:, :, n0:n0 + 128], c18h[:],
                                start=False, stop=False,
                                perf_mode=mybir.MatmulPerfMode.DoubleRow,
                                skip_group_check=seeded)
                            nc.tensor.matmul(
                                psS[:, h, :], z28[:, :, n0:n0 + 128], c18l[:],
                                start=False, stop=True,
                                perf_mode=mybir.MatmulPerfMode.DoubleRow,
                                skip_group_check=seeded)

                    # S = dots + bias (SBUF copy) and row-max, in one DVE op;
                    # then the argmax index of each row via max_index
                    if tail == "t0":
                        continue
                    if tail == "nottr":
                        nc.scalar.copy(out_sb[:, 2 * p:2 * p + 2, :],
                                       psS[:, :, 0:D])
                        continue
                    S_sb = sco.tile([128, 2, K], f32, tag="S")
                    mxt = sco.tile([128, 2, 8], f32, tag="mx")
                    post_done = False
                    if mx_bcast == "memset":
                        mseng.memset(mxt[:], -3.0e38)
                    if fuse in ("ts_seed", "mmbias") and post == "actcp":
                        # ONE ACT copy per pair (FD=1024): ScalarE pays a
                        # large per-op bubble on HW, so 128 per-h copies
                        # measured +79us vs 64 pair copies.  DVE max runs in
                        # 2x mode on SBUF operands (PSUM reads are 1x); out
                        # goes to a bf16 trash tile (not in-place) to avoid
                        # same-address read/write pipe hazards.
                        nc.scalar.copy(S_sb[:], psS[:])
                        for h in range(2):
                            meng = nc.gpsimd if (gp_split and h == 0) \
                                else nc.vector
                            meng.tensor_scalar(
                                out=strash[:, h, :], in0=S_sb[:, h, :],
                                scalar1=0.0, scalar2=None, op0=Alu.add,
                                op1=Alu.max, accum_out=mxt[:, h, 0:1])
                    elif fuse in ("ts_seed", "mmbias"):
                        # psS already biased; copy to SBUF + row max in one op
                        for h in range(2):
                            nc.vector.tensor_scalar(
                                out=S_sb[:, h, :], in0=psS[:, h, :],
                                scalar1=0.0, scalar2=None, op0=Alu.add,
                                op1=Alu.max, accum_out=mxt[:, h, 0:1])
                    elif fuse == "ttr_sbuf":
                        Snb = sco.tile([128, 2, K], f32, tag="Snb")
                        nc.scalar.copy(Snb[:], psS[:])
                        for h in range(2):
                            nc.vector.tensor_tensor_reduce(
                                out=S_sb[:, h, :], in0=Snb[:, h, :],
                                in1=bias[:, h, :], scale=1.0, scalar=-3.0e38,
                                op0=Alu.add, op1=Alu.max,
                                accum_out=mxt[:, h, 0:1])
                    else:
                        for h in range(2):
                            nc.vector.tensor_tensor_reduce(
                                out=S_sb[:, h, :], in0=psS[:, h, :],
                                in1=bias[:, h, :], scale=1.0, scalar=-3.0e38,
                                op0=Alu.add, op1=Alu.max,
                                accum_out=mxt[:, h, 0:1])
                    if tail == "t1":
                        continue
                    if tail == "nomax":
                        nc.scalar.copy(out_sb[:, 2 * p:2 * p + 2, :],
                                       S_sb[:, :, 0:D])
                        continue
                    if not post_done:
                        for h in range(2):
                            in_max = (mxt[:, h, 0:1].broadcast_to([128, 8])
                                      if mx_bcast == "stride0"
                                      else mxt[:, h, :])
                            nc.vector.max_index(out=idx8t[:, 2 * p + h, :],
                                                in_max=in_max,
                                                in_values=S_sb[:, h, :])
                    if tail == "t2":
                        continue

                    if (tailq and is_last and tail in ("full", "t4")
                            and p % 4 == 3):
                        qq = p // 4
                        ts = slice(8 * qq, 8 * (qq + 1))
                        for pp in range(8):
                            s8 = idx8t[16 * pp:16 * (pp + 1), ts, 0:1] \
                                .bitcast(i16)
                            weng = weng2 if pp % 2 else nc.sync
                            if wrapmode in ("both", "hw"):
                                weng.dma_start(idxw[16:32, ts, pp:pp + 1], s8)
                            if wrapmode in ("both", "sim"):
                                weng.dma_start(idxw[0:16, ts, pp:pp + 1], s8)
                        nc.gpsimd.dma_gather(
                            out_sb[:, ts, :], cbg_ext[b], idxw[:, ts, :],
                            num_idxs=N // 4, num_idxs_reg=N // 4, elem_size=D)
                        oeng = nc.sync if qq % 2 else weng2
                        oeng.dma_start(out_ext[b][:, ts], out_sb[:, ts])

                if (not (tailq and is_last)
                        and tail in ("full", "fulldbg", "nogather", "t3",
                                     "t4")):
                    # wrap: idxs[i%16, i//16] = idx of position i
                    # (i = t*128 + pk  ->  [pk%16, t*8 + pk//16]).
                    # The SWDGE queue-0 descgen core reads partitions 16:32
                    # (measured); CoreSim reads 0:16 — write both copies
                    # directly from idx8t.
                    for pp in range(8):
                        s8 = idx8t[16 * pp:16 * (pp + 1), :, 0:1].bitcast(i16)
                        weng = weng2 if pp % 2 else nc.sync
                        if wrapmode in ("both", "hw"):
                            weng.dma_start(idxw[16:32, :, pp:pp + 1], s8)
                        if wrapmode in ("both", "sim"):
                            weng.dma_start(idxw[0:16, :, pp:pp + 1], s8)
                    if tail in ("full", "fulldbg", "t4"):
                        # dma_gather caps out between 1024 and 2048 idxs
                        # per instruction (measured) — split in 4
                        for qq in range(4):
                            ts = slice(8 * qq, 8 * (qq + 1))
                            nc.gpsimd.dma_gather(
                                out_sb[:, ts, :], cbg_ext[b],
                                idxw[:, ts, :], num_idxs=N // 4,
                                num_idxs_reg=N // 4, elem_size=D)

                if tail in ("t0", "t1", "t2", "t3"):
                    # same-size dump of z1 (out_sb is never written here)
                    teng.dma_start(out_ext[b], z1[:])
                elif tailq and is_last and tail in ("full", "t4"):
                    pass  # emitted per quarter inside the p loop
                elif tail in ("full", "fulldbg", "t4"):
                    hn = nt // 2
                    nc.sync.dma_start(out_ext[b][:, 0:hn], out_sb[:, 0:hn])
                    weng2.dma_start(out_ext[b][:, hn:nt], out_sb[:, hn:nt])
                    if tail == "fulldbg":
                        teng.dma_start(dbg_ext[b], idxw[:])
                elif tail in ("nottr", "nomax"):
                    teng.dma_start(out_ext[b], out_sb[:])
                elif tail == "nowrap":
                    teng.dma_start(out_ext[b][:, :, 0:8].bitcast(u16),
                                   idx8t[:, :, :])
                else:  # nogather
                    teng.dma_start(out_ext[b][:, :, 0:8].bitcast(i16),
                                   idxw[:, :, :])

    nc.compile()
    return nc


def _get_nc():
    if "nc" not in _CACHE:
        _CACHE["nc"] = _build(fuse=FUSE)
    return _CACHE["nc"]


def _prep_in_maps(z_e_x, c, embedding, scheme=None, bpc=BPC,
                  fuse="mmbias", bias_dr=None):
    if scheme is None:
        scheme = SCHEME
    if bias_dr is None:
        bias_dr = BIAS_DR
    import ml_dtypes

    bf = ml_dtypes.bfloat16
    f8 = ml_dtypes.float8_e4m3

    z = np.ascontiguousarray(np.asarray(z_e_x), dtype=np.float32)      # [B, D, H, W]
    cls = np.asarray(c).astype(np.int64)                               # [B]
    emb = np.ascontiguousarray(np.asarray(embedding), dtype=np.float32)

    def dchunk(a):  # [B, 256, X] -> [B, 128, 2, X] with d = cd*128 + p
        return np.ascontiguousarray(
            a.reshape(B, 2, 128, a.shape[-1]).transpose(0, 2, 1, 3))

    zf = z.reshape(B, D, N)                                            # [B, 256, 4096]
    z1 = zf.astype(bf)
    z2f = zf - z1.astype(np.float32)

    cb = emb.reshape(NUM_CLASSES, K, D)[cls]                           # [B, 512, 256]
    cbT = np.ascontiguousarray(cb.transpose(0, 2, 1))                  # [B, 256, 512]
    c1 = cbT.astype(bf)
    c2f = cbT - c1.astype(np.float32)

    scale = (np.float32(65536.0) if scheme in ("fp8c", "fp8b", "fp8m")
             else np.float32(1.0))

    bias = -0.5 * np.sum(cb.astype(np.float64) ** 2, axis=2)           # [B, 512]
    bias_s = (bias * np.float64(scale)).astype(np.float32)
    if fuse == "mmbias" and bias_dr:
        assert scale > 1.0, "bias_dr needs the fp8 2^16 scaling"
        terms = []
        r = bias_s.astype(np.float32).copy()
        for _ in range(4):
            t = (r / np.float32(64.0)).astype(f8)
            terms.append(t)
            r = r - np.float32(64.0) * t.astype(np.float32)
        # [B, 2(part), 2(pair), K]
        bias4 = np.ascontiguousarray(
            np.stack(terms, axis=1).reshape(B, 2, 2, K))
    elif fuse == "mmbias":
        bhi = bias_s.astype(bf)
        blo = (bias_s - bhi.astype(np.float32)).astype(bf)
        # [B, 2(hi/lo), 2(h), K]
        bias2 = np.ascontiguousarray(
            np.stack([bhi, blo], axis=1)[:, :, None, :].repeat(2, axis=2))
    else:
        bias_bc = np.ascontiguousarray(np.broadcast_to(
            bias_s[:, None, None, :], (B, 128, 2, K)))

    cbg = cb.astype(bf)                                                # [B, 512, 256]

    per = {
        "z1": dchunk(z1),
        "ct1": dchunk((c1.astype(np.float32) * scale).astype(bf)),
        "cbg": cbg,
    }
    if fuse == "mmbias" and bias_dr:
        per["bias4"] = bias4
    elif fuse == "mmbias":
        per["bias2"] = bias2
    else:
        per["bias"] = bias_bc
    if scheme in ("exact3", "fp8c"):
        per["z2"] = dchunk(z2f.astype(bf))
    if scheme == "exact3":
        per["ct2"] = dchunk(c2f.astype(bf))
    if scheme in ("fp8c", "fp8b", "fp8m"):
        per["z8"] = dchunk((zf * np.float32(16.0)).astype(f8))
        per["c28"] = dchunk((c2f * np.float32(4096.0)).astype(f8))
    if scheme in ("fp8b", "fp8m"):
        per["z28"] = dchunk((z2f * np.float32(256.0)).astype(f8))
    if scheme == "fp8b":
        per["c18"] = dchunk((c1.astype(np.float32) * np.float32(256.0)).astype(f8))
    if scheme == "fp8m":
        c1s = c1.astype(np.float32) * np.float32(256.0)
        c18h = c1s.astype(f8)
        c18l = (c1s - c18h.astype(np.float32)).astype(f8)
        per["c18h"] = dchunk(c18h)
        per["c18l"] = dchunk(c18l)

    in_maps = []
    for i in range(NCORES):
        s = slice(i * bpc, (i + 1) * bpc)
        in_maps.append({k: v[s] for k, v in per.items()})
    return in_maps


FUSE = "mmbias"
BIAS_DR = 0


def kernel(z_e_x, c, embedding):
    from concourse.bass_utils import run_bass_kernel_spmd

    global LAST_EXEC_NS

    in_maps = _prep_in_maps(z_e_x, c, embedding, fuse=FUSE)
    nc = _get_nc()
    res = run_bass_kernel_spmd(nc, in_maps, core_ids=list(range(NCORES)),
                               trace=TRACE)
    LAST_EXEC_NS = res.exec_time_ns

    outs = np.concatenate([res.results[i]["out"].astype(np.float32)
                           for i in range(NCORES)], axis=0)
    # [B, 128, NT, D] -> [B, N, D] with n = t*128 + p
    out = outs.transpose(0, 2, 1, 3).reshape(B, N, D)
    return np.ascontiguousarray(out.reshape(B, HH, WW, D))

